# revision 3
# baseline (speedup 1.0000x reference)
"""Trainium2 Bass kernel for nn_AlternateLayer: stacked hidden-size-1 LSTMs.

Math (matching the jax reference):
  N = B*S = 2048 sequences. Per sequence: xf = flip(x, -1).reshape(T=30, 500).
  Layer 0: pre0[t] = xf[t] @ w_ih0.T + b_ih0 + b_hh0  (the only GEMM),
  then 64 stacked LSTM layers of hidden size 1 (layers 1..63 take the scalar
  h-stream of the layer below as input).

Implementation (v3):
  - Pure data parallelism: 256 sequences per NeuronCore (8 cores), split into
    NSTREAM=2 independent batch streams of NW=128 for chain-latency hiding.
  - Wavefront over (layer l, time t): step s processes layers l with l+t=s,
    93 steps. Per stream-step the engine work is minimized:
      * 2 band matmuls (K=65: 64 state rows holding 2h + ones/bias row) into
        one PSUM tile G[128, 2NW]; gate row layout pair0=(f 0:64, i 64:128),
        pair1=(o 0:64, g 64:128) so every DVE op is partition-aligned.
      * ONE merged tanh ACT Y = tanh(G) [128, 2NW] (all-tanh trick: f,i,o
        preacts pre-scaled x0.5 in the band so sigma folds into tanh).
      * DVE: v = (Yi+1)*Yg -> UV[64+sl]; u = (Yf+1)*C2prev -> UV[sl];
        H2' = (Yo+1)*TC -> H[sl].
      * C2' = 0.5u + v via a constant matmul Mcomb (cross-partition combine
        that DVE cannot do) -> PSUM C[64, 2NW] shared by both streams.
      * ONE merged tc ACT TC = tanh(0.5*C2') [64, 2NW] for both streams.
  - pre0 (layer-0 x-contribution) is computed by 15 x-GEMMs [4, 512] into
    PSUM, copied once to an SBUF strip PRE0[4, T*NPC], and injected into G
    by a K=4 accumulating matmul for steps s<=29 (no per-step ScalarE copy).
  - Output h_63(t) staged by GpSimd copies into a partition-63 strip and
    DMA'd out in 3 chunks.
  - States use the doubled convention (rows hold 2h / 2c); host halves out.
"""

import sys

sys.path.insert(0, "/opt/trn_rl_repo")

import numpy as np

import concourse.bacc as bacc
import concourse.bass as bass
import concourse.mybir as mybir
import concourse.tile as tile
from concourse.bass_utils import run_bass_kernel_spmd

B, S, T, D = 32, 64, 30, 500
L = 64
NCORES = 8
NPC = (B * S) // NCORES  # 256 sequences per core
DP = 512  # padded D
NSTEPS = L + T - 1  # 93
NSTREAM = 2
NW = NPC // NSTREAM  # 128
KH = 65  # band contraction: 64 state rows + ones row
BF16 = mybir.dt.np(mybir.dt.bfloat16)

# torch gate order for PRE0 rows / weight indexing: 0=i, 1=f, 2=g, 3=o
ARGSC = [0.5, 0.5, 1.0, 0.5]  # sigma-arg halving (not for g)
# G row layout: (pair, half) -> torch gate: pair0=(f,i), pair1=(o,g)
PAIRGATE = {(0, 0): 1, (0, 1): 0, (1, 0): 3, (1, 1): 2}

TC_MERGED = True  # one tanh(c) ACT for both streams
PCOPY_ENGINE = "scalar"  # P->PRE0 copies: "scalar" | "vector" | "gpsimd"
STAGE_ENGINE = "gpsimd"  # out staging: "gpsimd" | "vector"

_CACHE = {}


def _build_program():
    nc = bacc.Bacc(
        "TRN2",
        target_bir_lowering=False,
        debug=False,
        enable_asserts=False,
        num_devices=NCORES,
    )
    f32 = mybir.dt.float32
    bf16 = mybir.dt.bfloat16
    ACT_TANH = mybir.ActivationFunctionType.Tanh
    ACT_COPY = mybir.ActivationFunctionType.Copy
    MUL = mybir.AluOpType.mult
    ADD = mybir.AluOpType.add

    xt_d = nc.dram_tensor("xt", [DP, T * NPC], bf16, kind="ExternalInput").ap()
    wg_d = nc.dram_tensor("wg", [4, 128, 4], bf16, kind="ExternalInput").ap()
    wband_d = nc.dram_tensor("wband", [KH, 2, 128], bf16, kind="ExternalInput").ap()
    mcomb_d = nc.dram_tensor("mcomb", [128, 64], bf16, kind="ExternalInput").ap()
    wpre_d = nc.dram_tensor("wpre", [4, 2, 128], bf16, kind="ExternalInput").ap()
    ones_d = nc.dram_tensor("onesrow", [1, NPC], bf16, kind="ExternalInput").ap()
    out_d = nc.dram_tensor("out", [1, T * NPC], bf16, kind="ExternalOutput").ap()

    with tile.TileContext(nc) as tc:
        import contextlib

        with contextlib.ExitStack() as ctx:
            consts = ctx.enter_context(tc.tile_pool(name="consts", bufs=1))
            xpool = ctx.enter_context(tc.tile_pool(name="x", bufs=1))
            state = ctx.enter_context(tc.tile_pool(name="state", bufs=1))
            ypool = ctx.enter_context(tc.tile_pool(name="y", bufs=2))
            tcpool = ctx.enter_context(tc.tile_pool(name="tc", bufs=2))
            gpool = ctx.enter_context(tc.tile_pool(name="g", bufs=2, space="PSUM"))
            cpool = ctx.enter_context(tc.tile_pool(name="c", bufs=2, space="PSUM"))
            ppool = ctx.enter_context(tc.tile_pool(name="p", bufs=2, space="PSUM"))

            wband = consts.tile([KH, 2, 128], bf16)
            nc.sync.dma_start(wband[:], wband_d[:])
            mcomb = consts.tile([128, 64], bf16)
            nc.sync.dma_start(mcomb[:], mcomb_d[:])
            wpre = consts.tile([4, 2, 128], bf16)
            nc.sync.dma_start(wpre[:], wpre_d[:])
            wg = []
            for c in range(4):
                w = consts.tile([128, 4], bf16, tag=f"wg{c}", name=f"wg{c}")
                nc.sync.dma_start(w[:], wg_d[c])
                wg.append(w)

            # x DMA in groups (first small so GEMM 0 starts early)
            GROUPS = [2, 4, 6, 6, 6, 6]
            xt = []
            for c in range(4):
                xt.append(xpool.tile([128, T * NPC], bf16, tag=f"xt{c}", name=f"xt{c}"))
            t0 = 0
            for gsz in GROUPS:
                cs, ce = t0 * NPC, (t0 + gsz) * NPC
                for c in range(4):
                    nc.sync.dma_start(
                        xt[c][:, cs:ce], xt_d[c * 128 : (c + 1) * 128, cs:ce]
                    )
                t0 += gsz

            # persistent state tiles
            Hs, UVs = [], []
            for q in range(NSTREAM):
                Hq = state.tile([KH, NW], bf16, tag=f"H{q}", name=f"H{q}")
                UVq = state.tile([128, NW], bf16, tag=f"UV{q}", name=f"UV{q}")
                nc.vector.memset(Hq[:], 0.0)
                nc.vector.memset(UVq[:], 0.0)
                nc.sync.dma_start(Hq[64:65, :], ones_d[:, 0:NW])
                Hs.append(Hq)
                UVs.append(UVq)
            outbuf = state.tile([64, T * NPC], bf16, tag="outbuf", name="outbuf")

            # layer-0 x-GEMMs: P[4, 512] per 2 timesteps; copy to PRE0 strip
            pre0 = state.tile([4, T * NPC], bf16, tag="pre0", name="pre0")
            NPAIR = T // 2
            for p in range(NPAIR):
                P = ppool.tile([4, 2 * NPC], f32, tag="P", name="P")
                for c in range(4):
                    nc.tensor.matmul(
                        P[:],
                        wg[c][:],
                        xt[c][:, (2 * p) * NPC : (2 * p + 2) * NPC],
                        start=(c == 0),
                        stop=(c == 3),
                    )
                dst = pre0[:, (2 * p) * NPC : (2 * p + 2) * NPC]
                if PCOPY_ENGINE == "scalar":
                    nc.scalar.activation(dst, P[:], ACT_COPY)
                elif PCOPY_ENGINE == "vector":
                    nc.vector.tensor_copy(dst, P[:])
                else:
                    nc.gpsimd.tensor_copy(dst, P[:])

            # initial C2' (= zeros) for step 0
            Cprev = cpool.tile([64, 2 * NW], f32, tag="C", name="C")
            nc.vector.memset(Cprev[:], 0.0)

            # --- wavefront ---
            for s in range(NSTEPS):
                lo = max(0, s - (T - 1))
                hi = min(L - 1, s)
                a0 = 32 * (lo // 32)
                sl = slice(a0, hi + 1)
                slh = slice(64 + a0, 64 + hi + 1)

                Ys = []
                for q in range(NSTREAM):
                    H, UV = Hs[q], UVs[q]
                    G = gpool.tile([128, 2 * NW], f32, tag=f"G{q}", name=f"G{q}")
                    for p in range(2):
                        gslice = G[:, p * NW : (p + 1) * NW]
                        nc.tensor.matmul(
                            gslice,
                            wband[0:KH, p],
                            H[0:KH],
                            start=True,
                            stop=(s > T - 1),
                        )
                        if s <= T - 1:
                            nc.tensor.matmul(
                                gslice,
                                wpre[0:4, p],
                                pre0[0:4, s * NPC + q * NW : s * NPC + (q + 1) * NW],
                                start=False,
                                stop=True,
                            )
                    Y = ypool.tile([128, 2 * NW], bf16, tag=f"Y{q}", name=f"Y{q}")
                    nc.scalar.activation(Y[:], G[:], ACT_TANH)
                    Ys.append(Y)
                    # v = (Yi + 1) * Yg   (rows 64+sl; Yi pair0-half1, Yg pair1-half1)
                    nc.vector.scalar_tensor_tensor(
                        UV[slh], Y[slh, 0:NW], 1.0, Y[slh, NW : 2 * NW],
                        op0=ADD, op1=MUL,
                    )
                    # u = (Yf + 1) * C2prev   (rows sl; C2prev is PSUM f32)
                    nc.vector.scalar_tensor_tensor(
                        UV[sl], Y[sl, 0:NW], 1.0,
                        Cprev[sl, q * NW : (q + 1) * NW],
                        op0=ADD, op1=MUL,
                    )

                # C2' = 0.5*u + v via constant matmul (both streams into one tile)
                Cnew = cpool.tile([64, 2 * NW], f32, tag="C", name="C")
                for q in range(NSTREAM):
                    nc.tensor.matmul(
                        Cnew[:, q * NW : (q + 1) * NW],
                        mcomb[:],
                        UVs[q][0:128],
                        start=True,
                        stop=True,
                    )
                # TC = tanh(0.5 * C2') for both streams
                TC = tcpool.tile([64, 2 * NW], bf16, tag="TC", name="TC")
                nc.scalar.activation(TC[:], Cnew[:], ACT_TANH, scale=0.5)

                for q in range(NSTREAM):
                    # H2' = (Yo + 1) * TC   (rows sl; Yo pair1-half0)
                    nc.vector.scalar_tensor_tensor(
                        Hs[q][sl], Ys[q][sl, NW : 2 * NW], 1.0,
                        TC[sl, q * NW : (q + 1) * NW],
                        op0=ADD, op1=MUL,
                    )
                    if s >= L - 1:
                        t = s - (L - 1)
                        cc = slice(t * NPC + q * NW, t * NPC + (q + 1) * NW)
                        if STAGE_ENGINE == "gpsimd":
                            # gpsimd needs aligned partition blocks; copying
                            # rows 32:64 costs the same (free-dim iterations)
                            nc.gpsimd.tensor_copy(
                                outbuf[32:64, cc], Hs[q][32:64, :]
                            )
                        else:
                            nc.vector.tensor_copy(
                                outbuf[63:64, cc], Hs[q][63:64, :]
                            )
                Cprev = Cnew

                t = s - (L - 1)
                if t in (9, 19, 29):
                    c0 = (t - 9) * NPC
                    c1 = (t + 1) * NPC
                    nc.sync.dma_start(out_d[0:1, c0:c1], outbuf[63:64, c0:c1])

    nc.compile()
    return nc


def _prep_core_inputs(x_shard, w_ih0, w_hh0, b_ih0, b_hh0, w_ih, w_hh, b_ih, b_hh):
    """Host-side prep of one core's input arrays."""
    xr = x_shard[:, ::-1].astype(np.float32)  # [NPC, 15000]
    xr = np.ascontiguousarray(xr).reshape(NPC, T, D)
    xp = np.zeros((NPC, T, DP), dtype=np.float32)
    xp[:, :, :D] = xr
    xt = np.ascontiguousarray(xp.transpose(2, 1, 0).reshape(DP, T * NPC))
    xt = xt.astype(BF16)

    # wg[c][d, r]: PRE0 row r (torch gate r) = argsc_r * (w_ih0[r] . x)
    wg = np.zeros((4, 128, 4), dtype=np.float32)
    for r in range(4):
        wcol = np.zeros(DP, dtype=np.float32)
        wcol[:D] = w_ih0[r, :] * ARGSC[r]
        for c in range(4):
            wg[c, :, r] = wcol[c * 128 : (c + 1) * 128]

    # wband[k, pair, m]: k in 0:64 state rows (2h), k=64 ones row
    wband = np.zeros((KH, 2, 128), dtype=np.float32)
    for pair in range(2):
        for half in range(2):
            tg = PAIRGATE[(pair, half)]
            argsc = ARGSC[tg]
            hsc = 0.5 * argsc  # state rows hold 2h
            for l in range(L):
                m = 64 * half + l
                if l == 0:
                    wband[0, pair, m] = hsc * w_hh0[tg, 0]
                    wband[64, pair, m] = argsc * (b_ih0[tg] + b_hh0[tg])
                else:
                    wband[l - 1, pair, m] = hsc * w_ih[l - 1, tg, 0]
                    wband[l, pair, m] = hsc * w_hh[l - 1, tg, 0]
                    wband[64, pair, m] = argsc * (b_ih[l - 1, tg] + b_hh[l - 1, tg])

    # mcomb[k, m]: C2'[m] = 0.5*UV[m] + UV[64+m]
    mcomb = np.zeros((128, 64), dtype=np.float32)
    for m in range(64):
        mcomb[m, m] = 0.5
        mcomb[64 + m, m] = 1.0

    # wpre[r, pair, m]: inject PRE0 row r into layer-0 gate cols
    wpre = np.zeros((4, 2, 128), dtype=np.float32)
    for pair in range(2):
        for half in range(2):
            tg = PAIRGATE[(pair, half)]
            wpre[tg, pair, 64 * half + 0] = 1.0

    return {
        "xt": xt,
        "wg": wg.astype(BF16),
        "wband": wband.astype(BF16),
        "mcomb": mcomb.astype(BF16),
        "wpre": wpre.astype(BF16),
        "onesrow": np.ones((1, NPC), dtype=BF16),
    }


def _run(inputs, trace=False, trace_kwargs=None):
    if "nc" not in _CACHE:
        _CACHE["nc"] = _build_program()
    nc = _CACHE["nc"]

    x = np.asarray(inputs["x"], dtype=np.float32).reshape(B * S, T * D)
    params = {
        k: np.asarray(inputs[k], dtype=np.float32)
        for k in ("w_ih0", "w_hh0", "b_ih0", "b_hh0", "w_ih", "w_hh", "b_ih", "b_hh")
    }
    in_maps = []
    for i in range(NCORES):
        shard = x[i * NPC : (i + 1) * NPC]
        in_maps.append(_prep_core_inputs(shard, **params))

    res = run_bass_kernel_spmd(
        nc,
        in_maps,
        core_ids=list(range(NCORES)),
        trace=trace,
        **(trace_kwargs or {}),
    )

    out = np.empty((B * S, T), dtype=np.float32)
    for i in range(NCORES):
        # device stores H = 2h (doubled state); halve on the host
        o = np.asarray(res.results[i]["out"]).astype(np.float32)
        out[i * NPC : (i + 1) * NPC] = o.reshape(T, NPC).T * 0.5
    return out.reshape(B, S, T), res


def kernel(**inputs):
    out, _ = _run(inputs, trace=False)
    return out


# revision 8
# speedup vs baseline: 1.2333x; 1.2333x over previous
"""Trainium2 Bass kernel for nn_AlternateLayer: stacked hidden-size-1 LSTMs.

Math (matching the jax reference):
  N = B*S = 2048 sequences. Per sequence: xf = flip(x, -1).reshape(T=30, 500).
  Layer 0: pre0[t] = xf[t] @ w_ih0.T + b_ih0 + b_hh0  (the only GEMM),
  then 64 stacked LSTM layers of hidden size 1 (layers 1..63 take the scalar
  h-stream of the layer below as input).

Implementation (v3):
  - Pure data parallelism: 256 sequences per NeuronCore (8 cores), split into
    NSTREAM=2 independent batch streams of NW=128 for chain-latency hiding.
  - Wavefront over (layer l, time t): step s processes layers l with l+t=s,
    93 steps. Per stream-step the engine work is minimized:
      * 2 band matmuls (K=65: 64 state rows holding 2h + ones/bias row) into
        one PSUM tile G[128, 2NW]; gate row layout pair0=(f 0:64, i 64:128),
        pair1=(o 0:64, g 64:128) so every DVE op is partition-aligned.
      * ONE merged tanh ACT Y = tanh(G) [128, 2NW] (all-tanh trick: f,i,o
        preacts pre-scaled x0.5 in the band so sigma folds into tanh).
      * DVE: v = (Yi+1)*Yg -> UV[64+sl]; u = (Yf+1)*C2prev -> UV[sl];
        H2' = (Yo+1)*TC -> H[sl].
      * C2' = 0.5u + v via a constant matmul Mcomb (cross-partition combine
        that DVE cannot do) -> PSUM C[64, 2NW] shared by both streams.
      * ONE merged tc ACT TC = tanh(0.5*C2') [64, 2NW] for both streams.
  - pre0 (layer-0 x-contribution) is computed by 15 x-GEMMs [4, 512] into
    PSUM, copied once to an SBUF strip PRE0[4, T*NPC], and injected into G
    by a K=4 accumulating matmul for steps s<=29 (no per-step ScalarE copy).
  - Output h_63(t) staged by GpSimd copies into a partition-63 strip and
    DMA'd out in 3 chunks.
  - States use the doubled convention (rows hold 2h / 2c); host halves out.
"""

import sys

sys.path.insert(0, "/opt/trn_rl_repo")

import numpy as np

import concourse.bacc as bacc
import concourse.bass as bass
import concourse.mybir as mybir
import concourse.tile as tile
from concourse.bass_utils import run_bass_kernel_spmd

B, S, T, D = 32, 64, 30, 500
L = 64
NCORES = 8
NPC = (B * S) // NCORES  # 256 sequences per core
DP = 512  # padded D
NSTEPS = L + T - 1  # 93
NSTREAM = 2
NW = NPC // NSTREAM  # 128
KH = 65  # band contraction: 64 state rows + ones row
BF16 = mybir.dt.np(mybir.dt.bfloat16)

# torch gate order for PRE0 rows / weight indexing: 0=i, 1=f, 2=g, 3=o
ARGSC = [0.5, 0.5, 1.0, 0.5]  # sigma-arg halving (not for g)
# G row layout: (pair, half) -> torch gate: pair0=(f,i), pair1=(o,g)
PAIRGATE = {(0, 0): 1, (0, 1): 0, (1, 0): 3, (1, 1): 2}

PCOPY_ENGINE = "scalar"  # P->PRE0 copies: "scalar" | "vector" (gpsimd can't read PSUM)
STAGE_ENGINE = "gpsimd"  # out staging: "gpsimd" | "vector"

_CACHE = {}


def _build_program():
    nc = bacc.Bacc(
        "TRN2",
        target_bir_lowering=False,
        debug=False,
        enable_asserts=False,
        num_devices=NCORES,
    )
    f32 = mybir.dt.float32
    bf16 = mybir.dt.bfloat16
    ACT_TANH = mybir.ActivationFunctionType.Tanh
    ACT_COPY = mybir.ActivationFunctionType.Copy
    MUL = mybir.AluOpType.mult
    ADD = mybir.AluOpType.add

    xt_d = nc.dram_tensor("xt", [DP, T * NPC], bf16, kind="ExternalInput").ap()
    wg_d = nc.dram_tensor("wg", [4, 128, 4], bf16, kind="ExternalInput").ap()
    wband_d = nc.dram_tensor("wband", [KH, 2, 128], bf16, kind="ExternalInput").ap()
    mcomb_d = nc.dram_tensor("mcomb", [128, 64], bf16, kind="ExternalInput").ap()
    wpre_d = nc.dram_tensor("wpre", [4, 2, 128], bf16, kind="ExternalInput").ap()
    ones_d = nc.dram_tensor("onesrow", [1, NPC], bf16, kind="ExternalInput").ap()
    out_d = nc.dram_tensor("out", [1, T * NPC], bf16, kind="ExternalOutput").ap()

    with tile.TileContext(nc) as tc:
        import contextlib

        with contextlib.ExitStack() as ctx:
            consts = ctx.enter_context(tc.tile_pool(name="consts", bufs=1))
            xpool = ctx.enter_context(tc.tile_pool(name="x", bufs=1))
            state = ctx.enter_context(tc.tile_pool(name="state", bufs=1))
            ypool = ctx.enter_context(tc.tile_pool(name="y", bufs=2))
            tcpool = ctx.enter_context(tc.tile_pool(name="tc", bufs=2))
            # bufs=1: chain order guarantees Y-ACT(s) reads G before MM(s+1)
            gpool = ctx.enter_context(tc.tile_pool(name="g", bufs=1, space="PSUM"))
            cpool = ctx.enter_context(tc.tile_pool(name="c", bufs=2, space="PSUM"))
            ppool = ctx.enter_context(tc.tile_pool(name="p", bufs=2, space="PSUM"))

            wband = consts.tile([KH, 2, 128], bf16)
            nc.sync.dma_start(wband[:], wband_d[:])
            mcomb = consts.tile([128, 64], bf16)
            nc.sync.dma_start(mcomb[:], mcomb_d[:])
            wpre = consts.tile([4, 2, 128], bf16)
            nc.sync.dma_start(wpre[:], wpre_d[:])
            wg = []
            for c in range(4):
                w = consts.tile([128, 4], bf16, tag=f"wg{c}", name=f"wg{c}")
                nc.sync.dma_start(w[:], wg_d[c])
                wg.append(w)

            # x DMA in groups (first small so GEMM 0 starts early)
            GROUPS = [2, 4, 6, 6, 6, 6]
            xt = []
            for c in range(4):
                xt.append(xpool.tile([128, T * NPC], bf16, tag=f"xt{c}", name=f"xt{c}"))
            t0 = 0
            for gsz in GROUPS:
                cs, ce = t0 * NPC, (t0 + gsz) * NPC
                for c in range(4):
                    nc.sync.dma_start(
                        xt[c][:, cs:ce], xt_d[c * 128 : (c + 1) * 128, cs:ce]
                    )
                t0 += gsz

            # persistent state tiles
            Hs, UVs = [], []
            for q in range(NSTREAM):
                Hq = state.tile([KH, NW], bf16, tag=f"H{q}", name=f"H{q}")
                UVq = state.tile([128, NW], bf16, tag=f"UV{q}", name=f"UV{q}")
                nc.vector.memset(Hq[:], 0.0)
                nc.vector.memset(UVq[:], 0.0)
                nc.sync.dma_start(Hq[64:65, :], ones_d[:, 0:NW])
                Hs.append(Hq)
                UVs.append(UVq)
            outbuf = state.tile([64, T * NPC], bf16, tag="outbuf", name="outbuf")

            # layer-0 x-GEMMs: P[4, 512] per 2 timesteps; copy to PRE0 strip
            pre0 = state.tile([4, T * NPC], bf16, tag="pre0", name="pre0")
            NPAIR = T // 2
            for p in range(NPAIR):
                P = ppool.tile([4, 2 * NPC], f32, tag="P", name="P")
                for c in range(4):
                    nc.tensor.matmul(
                        P[:],
                        wg[c][:],
                        xt[c][:, (2 * p) * NPC : (2 * p + 2) * NPC],
                        start=(c == 0),
                        stop=(c == 3),
                    )
                dst = pre0[:, (2 * p) * NPC : (2 * p + 2) * NPC]
                if PCOPY_ENGINE == "scalar":
                    nc.scalar.activation(dst, P[:], ACT_COPY)
                elif PCOPY_ENGINE == "vector":
                    nc.vector.tensor_copy(dst, P[:])
                else:
                    nc.gpsimd.tensor_copy(dst, P[:])

            # initial C2' (= zeros) for step 0, per stream
            Cprev = []
            for q in range(NSTREAM):
                Cq = cpool.tile([64, NW], f32, tag=f"C{q}", name=f"C{q}")
                nc.vector.memset(Cq[:], 0.0)
                Cprev.append(Cq)

            # --- wavefront ---
            for s in range(NSTEPS):
                lo = max(0, s - (T - 1))
                hi = min(L - 1, s)
                a0 = 32 * (lo // 32)
                sl = slice(a0, hi + 1)
                slh = slice(64 + a0, 64 + hi + 1)

                for q in range(NSTREAM):
                    H, UV = Hs[q], UVs[q]
                    G = gpool.tile([128, 2 * NW], f32, tag=f"G{q}", name=f"G{q}")
                    for p in range(2):
                        gslice = G[:, p * NW : (p + 1) * NW]
                        nc.tensor.matmul(
                            gslice,
                            wband[0:KH, p],
                            H[0:KH],
                            start=True,
                            stop=(s > T - 1),
                        )
                        if s <= T - 1:
                            nc.tensor.matmul(
                                gslice,
                                wpre[0:4, p],
                                pre0[0:4, s * NPC + q * NW : s * NPC + (q + 1) * NW],
                                start=False,
                                stop=True,
                            )
                    Y = ypool.tile([128, 2 * NW], bf16, tag=f"Y{q}", name=f"Y{q}")
                    nc.scalar.activation(Y[:], G[:], ACT_TANH)
                    # v = (Yi + 1) * Yg   (rows 64+sl; Yi pair0-half1, Yg pair1-half1)
                    nc.vector.scalar_tensor_tensor(
                        UV[slh], Y[slh, 0:NW], 1.0, Y[slh, NW : 2 * NW],
                        op0=ADD, op1=MUL,
                    )
                    # u = (Yf + 1) * C2prev   (rows sl; C2prev is PSUM f32)
                    nc.vector.scalar_tensor_tensor(
                        UV[sl], Y[sl, 0:NW], 1.0, Cprev[q][sl, 0:NW],
                        op0=ADD, op1=MUL,
                    )
                    # C2' = 0.5*u + v via constant matmul (cross-partition fold)
                    Cnew = cpool.tile([64, NW], f32, tag=f"C{q}", name=f"C{q}")
                    nc.tensor.matmul(Cnew[:], mcomb[:], UV[0:128], start=True, stop=True)
                    # TC = tanh(0.5 * C2')
                    TC = tcpool.tile([64, NW], bf16, tag=f"TC{q}", name=f"TC{q}")
                    nc.scalar.activation(TC[:], Cnew[:], ACT_TANH, scale=0.5)
                    # H2' = (Yo + 1) * TC   (rows sl; Yo pair1-half0)
                    nc.vector.scalar_tensor_tensor(
                        H[sl], Y[sl, NW : 2 * NW], 1.0, TC[sl],
                        op0=ADD, op1=MUL,
                    )
                    Cprev[q] = Cnew
                    if s >= L - 1:
                        t = s - (L - 1)
                        cc = slice(t * NPC + q * NW, t * NPC + (q + 1) * NW)
                        if STAGE_ENGINE == "gpsimd":
                            # gpsimd needs aligned partition blocks; copying
                            # rows 32:64 costs the same (free-dim iterations)
                            nc.gpsimd.tensor_copy(outbuf[32:64, cc], H[32:64, :])
                        else:
                            nc.vector.tensor_copy(outbuf[63:64, cc], H[63:64, :])

                t = s - (L - 1)
                if t in (9, 19, 29):
                    c0 = (t - 9) * NPC
                    c1 = (t + 1) * NPC
                    nc.sync.dma_start(out_d[0:1, c0:c1], outbuf[63:64, c0:c1])

    nc.compile()
    return nc


def _prep_core_inputs(x_shard, w_ih0, w_hh0, b_ih0, b_hh0, w_ih, w_hh, b_ih, b_hh):
    """Host-side prep of one core's input arrays."""
    xr = x_shard[:, ::-1].astype(np.float32)  # [NPC, 15000]
    xr = np.ascontiguousarray(xr).reshape(NPC, T, D)
    xp = np.zeros((NPC, T, DP), dtype=np.float32)
    xp[:, :, :D] = xr
    xt = np.ascontiguousarray(xp.transpose(2, 1, 0).reshape(DP, T * NPC))
    xt = xt.astype(BF16)

    # wg[c][d, r]: PRE0 row r (torch gate r) = argsc_r * (w_ih0[r] . x)
    wg = np.zeros((4, 128, 4), dtype=np.float32)
    for r in range(4):
        wcol = np.zeros(DP, dtype=np.float32)
        wcol[:D] = w_ih0[r, :] * ARGSC[r]
        for c in range(4):
            wg[c, :, r] = wcol[c * 128 : (c + 1) * 128]

    # wband[k, pair, m]: k in 0:64 state rows (2h), k=64 ones row
    wband = np.zeros((KH, 2, 128), dtype=np.float32)
    for pair in range(2):
        for half in range(2):
            tg = PAIRGATE[(pair, half)]
            argsc = ARGSC[tg]
            hsc = 0.5 * argsc  # state rows hold 2h
            for l in range(L):
                m = 64 * half + l
                if l == 0:
                    wband[0, pair, m] = hsc * w_hh0[tg, 0]
                    wband[64, pair, m] = argsc * (b_ih0[tg] + b_hh0[tg])
                else:
                    wband[l - 1, pair, m] = hsc * w_ih[l - 1, tg, 0]
                    wband[l, pair, m] = hsc * w_hh[l - 1, tg, 0]
                    wband[64, pair, m] = argsc * (b_ih[l - 1, tg] + b_hh[l - 1, tg])

    # mcomb[k, m]: C2'[m] = 0.5*UV[m] + UV[64+m]
    mcomb = np.zeros((128, 64), dtype=np.float32)
    for m in range(64):
        mcomb[m, m] = 0.5
        mcomb[64 + m, m] = 1.0

    # wpre[r, pair, m]: inject PRE0 row r into layer-0 gate cols
    wpre = np.zeros((4, 2, 128), dtype=np.float32)
    for pair in range(2):
        for half in range(2):
            tg = PAIRGATE[(pair, half)]
            wpre[tg, pair, 64 * half + 0] = 1.0

    return {
        "xt": xt,
        "wg": wg.astype(BF16),
        "wband": wband.astype(BF16),
        "mcomb": mcomb.astype(BF16),
        "wpre": wpre.astype(BF16),
        "onesrow": np.ones((1, NPC), dtype=BF16),
    }


def _run(inputs, trace=False, trace_kwargs=None):
    if "nc" not in _CACHE:
        _CACHE["nc"] = _build_program()
    nc = _CACHE["nc"]

    x = np.asarray(inputs["x"], dtype=np.float32).reshape(B * S, T * D)
    params = {
        k: np.asarray(inputs[k], dtype=np.float32)
        for k in ("w_ih0", "w_hh0", "b_ih0", "b_hh0", "w_ih", "w_hh", "b_ih", "b_hh")
    }
    in_maps = []
    for i in range(NCORES):
        shard = x[i * NPC : (i + 1) * NPC]
        in_maps.append(_prep_core_inputs(shard, **params))

    res = run_bass_kernel_spmd(
        nc,
        in_maps,
        core_ids=list(range(NCORES)),
        trace=trace,
        **(trace_kwargs or {}),
    )

    out = np.empty((B * S, T), dtype=np.float32)
    for i in range(NCORES):
        # device stores H = 2h (doubled state); halve on the host
        o = np.asarray(res.results[i]["out"]).astype(np.float32)
        out[i * NPC : (i + 1) * NPC] = o.reshape(T, NPC).T * 0.5
    return out.reshape(B, S, T), res


def kernel(**inputs):
    out, _ = _run(inputs, trace=False)
    return out
